# revision 13
# baseline (speedup 1.0000x reference)
"""Causal multi-head attention on 8 Trainium2 NeuronCores.

Problem: x[2,2048,1024] @ W_Q/K/V[1024,1024] -> 16-head causal attention
(d_head=64) -> @ W_O[1024,1024].

Sharding: tensor-parallel over heads. Core i owns heads 2i, 2i+1 — i.e.
columns [128i:128i+128) of W_Q/W_K/W_V and rows [128i:128i+128) of W_O.
Each core computes its partial output [1024, 4096] (transposed layout, bf16);
the host sums the 8 partials and un-transposes (the "all-reduce").

Device kernel (per core), all matmul operands bf16 (PSUM accumulates fp32):
  1. Projections from xT [1024, 4096] (host pre-transposes + casts bf16):
     Q/K transposed [128, 4096] = W.T @ xT into a fused qkT tile;
     V directly in natural [token, dim] layout (x-chunk as the stationary
     operand), with a ones-column appended per head (65-wide blocks) so the
     PV matmul also produces the softmax denominator for free.
  2. Flash-style causal attention, scores in [k, q] orientation. The causal
     mask is applied INSIDE PSUM by an extra accumulating matmul that adds
     -1e9 above the diagonal (identity stationary x upper-tri mask), so exp
     on ScalarE needs no separate DVE mask multiply. exp is one packed
     [128, 2, live] instruction covering both heads.
  3. Normalization: denominator rows are batch-reciprocated [4, 512] per
     q-tile-pair (instead of 16 serial [1,512] reciprocals), broadcast via
     GPSIMD, and multiplied into bf16 attnT straight out of PSUM.
  4. Output projection interleaved per q-tile pair so the PE never idles
     (keeps the HAM clock-gate warm) and stores batch per q-tile.
"""

import contextlib

import ml_dtypes
import numpy as np

import concourse.bass as bass
import concourse.tile as tile
from concourse import bacc, mybir
from concourse.bass_utils import run_bass_kernel_spmd
from concourse.masks import make_identity

F32 = mybir.dt.float32
BF16 = mybir.dt.bfloat16

N_CORES = 8
P = 128
D = 1024          # d_model
B = 2             # batch
S = 2048          # seq len
T = B * S         # total tokens = 4096
TT = 512          # token tile (free dim of matmuls)
NT = T // TT      # 8 token tiles
KD = D // P       # 8 contraction chunks for projections
JB = S // TT      # 4 q-tiles per batch
CB = S // P       # 16 k-chunks per batch
NCH = T // P      # 32 k-chunks total
H_LOC = 2         # heads per core
DH = 64           # head dim


def _body(tc):
    nc = tc.nc
    xT = nc.dram_tensor("xT", [D, T], BF16, kind="ExternalInput").ap()
    wq = nc.dram_tensor("wq", [D, P], BF16, kind="ExternalInput").ap()
    wk = nc.dram_tensor("wk", [D, P], BF16, kind="ExternalInput").ap()
    wv = nc.dram_tensor("wv", [D, P], BF16, kind="ExternalInput").ap()
    wo = nc.dram_tensor("wo", [P, D], BF16, kind="ExternalInput").ap()
    outT = nc.dram_tensor("outT", [D, T], BF16, kind="ExternalOutput").ap()

    with contextlib.ExitStack() as ctx:
        const = ctx.enter_context(tc.tile_pool(name="const", bufs=1))
        wpool = ctx.enter_context(tc.tile_pool(name="wpool", bufs=1))
        xpool = ctx.enter_context(tc.tile_pool(name="xpool", bufs=2))
        persist = ctx.enter_context(tc.tile_pool(name="persist", bufs=1))
        probs_p = ctx.enter_context(tc.tile_pool(name="probs", bufs=4))
        stage = ctx.enter_context(tc.tile_pool(name="stage", bufs=2))
        bcp = ctx.enter_context(tc.tile_pool(name="bcp", bufs=4))
        obp = ctx.enter_context(tc.tile_pool(name="obp", bufs=2))
        psum = ctx.enter_context(tc.tile_pool(name="psum", bufs=2, space="PSUM"))

        # --- constants -----------------------------------------------------
        identity = const.tile([P, P], BF16)
        make_identity(nc, identity)

        # umask[k, q] = 0.0 if q >= k else -1e9 (added to scores pre-exp)
        umask = const.tile([P, P], BF16)
        nc.any.memset(umask[:], 0.0)
        nc.gpsimd.affine_select(
            out=umask[:],
            in_=umask[:],
            compare_op=mybir.AluOpType.is_ge,
            fill=-1e9,
            base=0,
            pattern=[[1, P]],
            channel_multiplier=-1,
        )

        # --- weights -------------------------------------------------------
        wq_sb = wpool.tile([P, KD, P], BF16)
        nc.sync.dma_start(wq_sb[:], wq.rearrange("(o p) m -> p o m", p=P))
        wk_sb = wpool.tile([P, KD, P], BF16)
        nc.sync.dma_start(wk_sb[:], wk.rearrange("(o p) m -> p o m", p=P))
        wv_sb = wpool.tile([P, KD, P], BF16)
        nc.sync.dma_start(wv_sb[:], wv.rearrange("(o p) m -> p o m", p=P))
        wo_sb = wpool.tile([P, D], BF16)
        nc.sync.dma_start(wo_sb[:], wo)

        # --- persistent activations ---------------------------------------
        qkT = persist.tile([P, 2, T], BF16)     # [:,0,:] = QT, [:,1,:] = KT
        vn = persist.tile([P, NCH, 130], BF16)  # [token, chunk, d0|1|d1|1]
        attnT = persist.tile([P, T], BF16)
        # memset (not an activation reading uninitialized SBUF: 0*NaN = NaN
        # would make results depend on leftover SBUF state across runs)
        for col in (DH, 2 * DH + 1):
            nc.any.memset(vn[:, :, col], 1.0)

        xT_r = xT.rearrange("(o p) n -> p o n", p=P)
        outT_r = outT.rearrange("(o p) n -> p o n", p=P)

        # --- phase 1 (per-tile helper, interleaved into the jj loop) -------
        def project(t):
            xt = xpool.tile([P, KD, TT], BF16, tag="xt", name=f"xt_{t}")
            nc.sync.dma_start(xt[:], xT_r[:, :, bass.ts(t, TT)])
            psqk = psum.tile([P, 2, TT], F32, tag="b", name=f"psqk_{t}")
            for g, wsb in ((0, wq_sb), (1, wk_sb)):
                for c in range(KD):
                    nc.tensor.matmul(psqk[:, g, :], wsb[:, c, :], xt[:, c, :],
                                     start=(c == 0), stop=(c == KD - 1))
            nc.vector.tensor_copy(qkT[:, :, bass.ts(t, TT)], psqk[:])
            # V: project transposed like Q/K (N=512 streams), then
            # PE-transpose into natural [token, dim] layout.
            psv = psum.tile([P, TT], F32, tag="b", name=f"psv_{t}")
            for c in range(KD):
                nc.tensor.matmul(psv[:], wv_sb[:, c, :], xt[:, c, :],
                                 start=(c == 0), stop=(c == KD - 1))
            vt = stage.tile([P, TT], BF16, tag="vt", name=f"vt_{t}")
            nc.scalar.copy(vt[:], psv[:])
            for s_ in range(4):
                ch = t * 4 + s_
                pt = psum.tile([P, P], BF16, tag="b", name=f"pt_{ch}")
                nc.tensor.transpose(pt[:], vt[:, bass.ts(s_, P)], identity)
                nc.vector.tensor_copy(
                    vn[:, ch, 0:130].rearrange("p (a b) -> p a b", a=2)
                    [:, :, 0:DH],
                    pt[:].rearrange("p (a b) -> p a b", a=2))

        # --- phase 2: causal attention + interleaved output projection ----
        # Dual-j: same-index q-tiles of batch 0/1 processed together. Lag-1
        # software pipeline: PV for chunk cb-1 is emitted after the scores
        # for chunk cb so the PE never waits on ScalarE's exp. On diagonal
        # chunks the causal mask is an accumulating matmul adding -1e9.
        def phase3(js):
            # output projection for two finished (normalized) q-tiles.
            # PSUM->SBUF casts alternate DVE/ScalarE so two are in flight
            # and the PE isn't gated on a single cast engine.
            for jx in js:
                jsl = bass.ts(jx, TT)
                ob = obp.tile([P, KD, TT], BF16, tag="ob", name=f"ob_{jx}")
                for fp_ in range(KD // 2):
                    wps = psum.tile([P, 2, TT], F32, tag="b",
                                    name=f"wps_{jx}_{fp_}")
                    for g in range(2):
                        f = fp_ * 2 + g
                        nc.tensor.matmul(wps[:, g, :],
                                         wo_sb[:, bass.ts(f, P)],
                                         attnT[:, jsl], start=True, stop=True)
                    dst = ob[:, fp_ * 2:fp_ * 2 + 2, :]
                    if fp_ % 2:
                        nc.scalar.copy(dst, wps[:])
                    else:
                        nc.vector.tensor_copy(dst, wps[:])
                nc.sync.dma_start(outT_r[:, :, jsl], ob[:])

        prev_js = None
        for jj in range(JB):
            # interleave the projections: tiles jj and jj+4 are exactly what
            # this jj's attention needs (q-tiles jj/jj+4; k-chunks <= tile jj)
            project(jj)
            project(jj + JB)
            ncb = 4 * (jj + 1)
            js = (jj, jj + JB)
            pvall = psum.tile([DH + 1, 4, TT], F32, tag="pv", bufs=1,
                              name=f"pv_{jj}")

            def pv_step(jx, cb, pr, jj=jj, ncb=ncb, pvall=pvall):
                b = jx // JB
                c = CB * b + cb
                r = cb - 4 * jj
                lo = P * r if r > 0 else 0
                jloc = 0 if jx == jj else 1
                for h in range(H_LOC):
                    nc.tensor.matmul(pvall[:, jloc * 2 + h, lo:],
                                     vn[:, c, bass.ds((DH + 1) * h, DH + 1)],
                                     pr[:, h, lo:],
                                     start=(cb == 0), stop=(cb == ncb - 1))

            pending = {}
            for cb in range(ncb):
                r = cb - 4 * jj
                lo = P * r if r > 0 else 0
                for jx in js:
                    b = jx // JB
                    c = CB * b + cb
                    csl = bass.ts(c, P)
                    jsl = bass.ts(jx, TT)
                    sps = psum.tile([P, 2, TT], F32, tag="b",
                                    name=f"sps_{jx}_{cb}")
                    for h in range(H_LOC):
                        hp = slice(DH * h, DH * h + DH)
                        nc.tensor.matmul(sps[:, h, lo:], qkT[hp, 1, csl],
                                         qkT[hp, 0, jsl][:, lo:],
                                         start=True, stop=(r < 0))
                    if r >= 0:
                        rsl = bass.ts(r, P)
                        for h in range(H_LOC):
                            nc.tensor.matmul(sps[:, h, rsl], identity[:],
                                             umask[:], start=False, stop=True)
                    pr = probs_p.tile([P, 2, TT], BF16, tag="pr",
                                      name=f"pr_{jx}_{cb}")
                    nc.scalar.activation(pr[:, :, lo:], sps[:, :, lo:],
                                         mybir.ActivationFunctionType.Exp,
                                         scale=0.125)
                    if jx in pending:
                        pv_step(jx, cb - 1, pending[jx])
                    pending[jx] = pr
            for jx in js:
                pv_step(jx, ncb - 1, pending[jx])

            # Quick pvall release: unnormalized bf16 casts into attnT plus
            # the denominator row, so the next jj's PV accumulation isn't
            # gated on the (long) reciprocal/broadcast chain.
            dnf = stage.tile([1, 4, TT], F32, tag="dnf", name=f"dnf_{jj}")
            nc.vector.tensor_copy(dnf[:], pvall[DH:DH + 1, :, :])
            for jloc, jx in enumerate(js):
                jsl = bass.ts(jx, TT)
                for h in range(H_LOC):
                    i = jloc * 2 + h
                    hp = slice(DH * h, DH * h + DH)
                    nc.vector.tensor_copy(attnT[hp, jsl], pvall[0:DH, i, :])

            # Deferred normalization (has a whole jj iteration of slack
            # before phase3 of this pair consumes attnT): batch-reciprocal
            # on 4 partitions, one packed partition-broadcast, in-place
            # bf16 multiplies.
            dn = stage.tile([4, TT], F32, tag="dn", name=f"dn_{jj}")
            nc.sync.dma_start(dn[:], dnf[:])
            rdn = stage.tile([4, TT], BF16, tag="rdn", name=f"rdn_{jj}")
            with nc.allow_low_precision(reason="bf16 1/denominator is ample"):
                nc.vector.reciprocal(rdn[:], dn[:])
            rf = stage.tile([1, 4, TT], BF16, tag="rf", name=f"rf_{jj}")
            nc.sync.dma_start(rf[:], rdn[:])
            bc4 = bcp.tile([P, 4, TT], BF16, tag="bc", bufs=2,
                           name=f"bc_{jj}")
            nc.gpsimd.partition_broadcast(bc4[:], rf[:])

            # Previous pair's output projection: ready PE work that fills
            # the gap while this jj's normalization drains on DVE/DMA/GPSIMD
            # (keeps the HAM clock-gate warm).
            if prev_js is not None:
                phase3(prev_js)

            for jloc, jx in enumerate(js):
                jsl = bass.ts(jx, TT)
                for h in range(H_LOC):
                    i = jloc * 2 + h
                    hp = slice(DH * h, DH * h + DH)
                    nc.vector.tensor_mul(attnT[hp, jsl], attnT[hp, jsl],
                                         bc4[hp, i, :])
            prev_js = js
        phase3(prev_js)


_NC_CACHE = None


def _get_nc():
    global _NC_CACHE
    if _NC_CACHE is None:
        nc = bacc.Bacc("TRN2", target_bir_lowering=False, debug=False,
                       num_devices=N_CORES)
        with tile.TileContext(nc) as tc:
            _body(tc)
        nc.compile()
        _NC_CACHE = nc
    return _NC_CACHE


def _in_maps(x, W_Q, W_K, W_V, W_O):
    bf16 = ml_dtypes.bfloat16
    xT = np.ascontiguousarray(
        np.asarray(x, dtype=np.float32).reshape(T, D).T).astype(bf16)
    W_Q = np.asarray(W_Q, dtype=np.float32).astype(bf16)
    W_K = np.asarray(W_K, dtype=np.float32).astype(bf16)
    W_V = np.asarray(W_V, dtype=np.float32).astype(bf16)
    W_O = np.asarray(W_O, dtype=np.float32).astype(bf16)
    maps = []
    for i in range(N_CORES):
        sl = slice(P * i, P * i + P)
        maps.append({
            "xT": xT,
            "wq": np.ascontiguousarray(W_Q[:, sl]),
            "wk": np.ascontiguousarray(W_K[:, sl]),
            "wv": np.ascontiguousarray(W_V[:, sl]),
            "wo": np.ascontiguousarray(W_O[sl, :]),
        })
    return maps


def _gather(results):
    acc = np.zeros([D, T], np.float32)
    for r in results:
        acc += np.asarray(r["outT"]).astype(np.float32)
    return np.ascontiguousarray(acc.T).reshape(B, S, D)


def kernel(x, W_Q, W_K, W_V, W_O):
    nc = _get_nc()
    res = run_bass_kernel_spmd(nc, _in_maps(x, W_Q, W_K, W_V, W_O),
                               core_ids=list(range(N_CORES)))
    return _gather(res.results)


def kernel_profiled(x, W_Q, W_K, W_V, W_O):
    """Like kernel() but with NTFF tracing; returns (output, exec_time_ns)."""
    nc = _get_nc()
    res = run_bass_kernel_spmd(nc, _in_maps(x, W_Q, W_K, W_V, W_O),
                               core_ids=list(range(N_CORES)), trace=True)
    return _gather(res.results), res.exec_time_ns


# revision 19
# speedup vs baseline: 1.1466x; 1.1466x over previous
"""Causal multi-head attention on 8 Trainium2 NeuronCores.

Problem: x[2,2048,1024] @ W_Q/K/V[1024,1024] -> 16-head causal attention
(d_head=64) -> @ W_O[1024,1024].

Sharding: tensor-parallel over heads. Core i owns heads 2i, 2i+1 — i.e.
columns [128i:128i+128) of W_Q/W_K/W_V and rows [128i:128i+128) of W_O.
Each core computes its partial output [1024, 4096] (transposed layout, bf16);
the host sums the 8 partials and un-transposes (the "all-reduce").

Device kernel (per core), all matmul operands bf16 (PSUM accumulates fp32):
  1. Projections from xT [1024, 4096] (host pre-transposes + casts bf16):
     Q/K transposed [128, 4096] = W.T @ xT into a fused qkT tile;
     V directly in natural [token, dim] layout (x-chunk as the stationary
     operand), with a ones-column appended per head (65-wide blocks) so the
     PV matmul also produces the softmax denominator for free.
  2. Flash-style causal attention, scores in [k, q] orientation. The causal
     mask is applied INSIDE PSUM by an extra accumulating matmul that adds
     -1e9 above the diagonal (identity stationary x upper-tri mask), so exp
     on ScalarE needs no separate DVE mask multiply. exp is one packed
     [128, 2, live] instruction covering both heads.
  3. Normalization: denominator rows are batch-reciprocated [4, 512] per
     q-tile-pair (instead of 16 serial [1,512] reciprocals), broadcast via
     GPSIMD, and multiplied into bf16 attnT straight out of PSUM.
  4. Output projection interleaved per q-tile pair so the PE never idles
     (keeps the HAM clock-gate warm) and stores batch per q-tile.
"""

import contextlib

import ml_dtypes
import numpy as np

import concourse.bass as bass
import concourse.tile as tile
from concourse import bacc, mybir
from concourse.bass_utils import run_bass_kernel_spmd


F32 = mybir.dt.float32
BF16 = mybir.dt.bfloat16

N_CORES = 8
P = 128
D = 1024          # d_model
B = 2             # batch
S = 2048          # seq len
T = B * S         # total tokens = 4096
TT = 512          # token tile (free dim of matmuls)
NT = T // TT      # 8 token tiles
KD = D // P       # 8 contraction chunks for projections
JB = S // TT      # 4 q-tiles per batch
CB = S // P       # 16 k-chunks per batch
NCH = T // P      # 32 k-chunks total
H_LOC = 2         # heads per core
DH = 64           # head dim


def _body(tc):
    nc = tc.nc
    xT = nc.dram_tensor("xT", [D, T], BF16, kind="ExternalInput").ap()
    wq = nc.dram_tensor("wq", [D, P], BF16, kind="ExternalInput").ap()
    wk = nc.dram_tensor("wk", [D, P], BF16, kind="ExternalInput").ap()
    wv = nc.dram_tensor("wv", [D, P], BF16, kind="ExternalInput").ap()
    wo = nc.dram_tensor("wo", [P, D], BF16, kind="ExternalInput").ap()
    outT = nc.dram_tensor("outT", [D, T], BF16, kind="ExternalOutput").ap()

    with contextlib.ExitStack() as ctx:
        const = ctx.enter_context(tc.tile_pool(name="const", bufs=1))
        wpool = ctx.enter_context(tc.tile_pool(name="wpool", bufs=1))
        xpool = ctx.enter_context(tc.tile_pool(name="xpool", bufs=2))
        persist = ctx.enter_context(tc.tile_pool(name="persist", bufs=1))
        probs_p = ctx.enter_context(tc.tile_pool(name="probs", bufs=4))
        stage = ctx.enter_context(tc.tile_pool(name="stage", bufs=2))
        bcp = ctx.enter_context(tc.tile_pool(name="bcp", bufs=4))
        obp = ctx.enter_context(tc.tile_pool(name="obp", bufs=2))
        psum = ctx.enter_context(tc.tile_pool(name="psum", bufs=2, space="PSUM"))

        # --- constants -----------------------------------------------------
        # mask_band[k, q] = 1.0 if q >= k else 0.0 (multiplies probs on the
        # diagonal chunk; cheap bf16 2x-mode DVE op, keeps the PE free)
        mask_band = const.tile([P, P], BF16)
        nc.any.memset(mask_band[:], 1.0)
        nc.gpsimd.affine_select(
            out=mask_band[:],
            in_=mask_band[:],
            compare_op=mybir.AluOpType.is_ge,
            fill=0.0,
            base=0,
            pattern=[[1, P]],
            channel_multiplier=-1,
        )

        # --- weights -------------------------------------------------------
        wq_sb = wpool.tile([P, KD, P], BF16)
        nc.sync.dma_start(wq_sb[:], wq.rearrange("(o p) m -> p o m", p=P))
        wk_sb = wpool.tile([P, KD, P], BF16)
        nc.sync.dma_start(wk_sb[:], wk.rearrange("(o p) m -> p o m", p=P))
        wv_sb = wpool.tile([P, KD, P], BF16)
        nc.sync.dma_start(wv_sb[:], wv.rearrange("(o p) m -> p o m", p=P))
        wo_sb = wpool.tile([P, D], BF16)
        nc.sync.dma_start(wo_sb[:], wo)

        # --- persistent activations ---------------------------------------
        qkT = persist.tile([P, 2, T], BF16)     # [:,0,:] = QT, [:,1,:] = KT
        vn = persist.tile([P, NCH, 130], BF16)  # [token, chunk, d0|1|d1|1]
        attnT = persist.tile([P, T], BF16)
        # memset (not an activation reading uninitialized SBUF: 0*NaN = NaN
        # would make results depend on leftover SBUF state across runs)
        for col in (DH, 2 * DH + 1):
            nc.any.memset(vn[:, :, col], 1.0)

        xT_r = xT.rearrange("(o p) n -> p o n", p=P)
        outT_r = outT.rearrange("(o p) n -> p o n", p=P)

        # --- phase 1 (per-tile helper, interleaved into the jj loop) -------
        def project(t):
            xt = xpool.tile([P, KD, TT], BF16, tag="xt", name=f"xt_{t}")
            nc.sync.dma_start(xt[:], xT_r[:, :, bass.ts(t, TT)])
            psqk = psum.tile([P, 2, TT], F32, tag="b", name=f"psqk_{t}")
            for g, wsb in ((0, wq_sb), (1, wk_sb)):
                for c in range(KD):
                    nc.tensor.matmul(psqk[:, g, :], wsb[:, c, :], xt[:, c, :],
                                     start=(c == 0), stop=(c == KD - 1))
            nc.vector.tensor_copy(qkT[:, :, bass.ts(t, TT)], psqk[:])
            # V: project transposed like Q/K (N=512 streams), then
            # PE-transpose into natural [token, dim] layout.
            psv = psum.tile([P, TT], F32, tag="b", name=f"psv_{t}")
            for c in range(KD):
                nc.tensor.matmul(psv[:], wv_sb[:, c, :], xt[:, c, :],
                                 start=(c == 0), stop=(c == KD - 1))
            vt = stage.tile([P, TT], BF16, tag="vt", name=f"vt_{t}")
            nc.scalar.copy(vt[:], psv[:])
            # transpose each 128-token chunk via the DMA xbar (idle engine)
            # instead of the PE, then split into the 65-wide head blocks.
            for s_ in range(4):
                ch = t * 4 + s_
                vtt = stage.tile([P, P], BF16, tag="vtt", name=f"vtt_{ch}")
                nc.sync.dma_start(vtt[:], vt[:, bass.ts(s_, P)],
                                  transpose=True)
                nc.vector.tensor_copy(
                    vn[:, ch, 0:130].rearrange("p (a b) -> p a b", a=2)
                    [:, :, 0:DH],
                    vtt[:].rearrange("p (a b) -> p a b", a=2))

        # --- phase 2: causal attention + interleaved output projection ----
        # Dual-j: same-index q-tiles of batch 0/1 processed together. Lag-1
        # software pipeline: PV for chunk cb-1 is emitted after the scores
        # for chunk cb so the PE never waits on ScalarE's exp. On diagonal
        # chunks the causal mask is an accumulating matmul adding -1e9.
        def phase3(js):
            # output projection for two finished (normalized) q-tiles.
            # PSUM->SBUF casts alternate DVE/ScalarE so two are in flight
            # and the PE isn't gated on a single cast engine.
            for jx in js:
                jsl = bass.ts(jx, TT)
                ob = obp.tile([P, KD, TT], BF16, tag="ob", name=f"ob_{jx}")
                for fp_ in range(KD // 2):
                    wps = psum.tile([P, 2, TT], F32, tag="b",
                                    name=f"wps_{jx}_{fp_}")
                    for g in range(2):
                        f = fp_ * 2 + g
                        nc.tensor.matmul(wps[:, g, :],
                                         wo_sb[:, bass.ts(f, P)],
                                         attnT[:, jsl], start=True, stop=True)
                    nc.vector.tensor_copy(ob[:, fp_ * 2:fp_ * 2 + 2, :],
                                          wps[:])
                nc.sync.dma_start(outT_r[:, :, jsl], ob[:])

        prev_js = None
        for jj in range(JB):
            # interleave the projections: tiles jj and jj+4 are exactly what
            # this jj's attention needs (q-tiles jj/jj+4; k-chunks <= tile jj)
            project(jj)
            project(jj + JB)
            ncb = 4 * (jj + 1)
            js = (jj, jj + JB)
            pvall = psum.tile([DH + 1, 4, TT], F32, tag="pv", bufs=1,
                              name=f"pv_{jj}")

            def pv_step(jx, cb, pr, jj=jj, ncb=ncb, pvall=pvall):
                b = jx // JB
                c = CB * b + cb
                r = cb - 4 * jj
                lo = P * r if r > 0 else 0
                jloc = 0 if jx == jj else 1
                for h in range(H_LOC):
                    nc.tensor.matmul(pvall[:, jloc * 2 + h, lo:],
                                     vn[:, c, bass.ds((DH + 1) * h, DH + 1)],
                                     pr[:, h, lo:],
                                     start=(cb == 0), stop=(cb == ncb - 1))

            pending = {}
            for cb in range(ncb):
                r = cb - 4 * jj
                lo = P * r if r > 0 else 0
                for jx in js:
                    b = jx // JB
                    c = CB * b + cb
                    csl = bass.ts(c, P)
                    jsl = bass.ts(jx, TT)
                    sps = psum.tile([P, 2, TT], F32, tag="b",
                                    name=f"sps_{jx}_{cb}")
                    for h in range(H_LOC):
                        hp = slice(DH * h, DH * h + DH)
                        nc.tensor.matmul(sps[:, h, lo:], qkT[hp, 1, csl],
                                         qkT[hp, 0, jsl][:, lo:],
                                         start=True, stop=True)
                    pr = probs_p.tile([P, 2, TT], BF16, tag="pr",
                                      name=f"pr_{jx}_{cb}")
                    nc.scalar.activation(pr[:, :, lo:], sps[:, :, lo:],
                                         mybir.ActivationFunctionType.Exp,
                                         scale=0.125)
                    if r >= 0:
                        rsl = bass.ts(r, P)
                        for h in range(H_LOC):
                            nc.vector.tensor_mul(pr[:, h, rsl],
                                                 pr[:, h, rsl], mask_band[:])
                    if jx in pending:
                        pv_step(jx, cb - 1, pending[jx])
                    pending[jx] = pr
            for jx in js:
                pv_step(jx, ncb - 1, pending[jx])

            # Quick pvall release: unnormalized bf16 casts into attnT plus
            # the denominator row, so the next jj's PV accumulation isn't
            # gated on the (long) reciprocal/broadcast chain.
            dnf = stage.tile([1, 4, TT], F32, tag="dnf", name=f"dnf_{jj}")
            nc.vector.tensor_copy(dnf[:], pvall[DH:DH + 1, :, :])
            for jloc, jx in enumerate(js):
                jsl = bass.ts(jx, TT)
                for h in range(H_LOC):
                    i = jloc * 2 + h
                    hp = slice(DH * h, DH * h + DH)
                    nc.vector.tensor_copy(attnT[hp, jsl], pvall[0:DH, i, :])

            # Deferred normalization (has a whole jj iteration of slack
            # before phase3 of this pair consumes attnT): batch-reciprocal
            # on 4 partitions, one packed partition-broadcast, in-place
            # bf16 multiplies.
            dn = stage.tile([4, TT], F32, tag="dn", name=f"dn_{jj}")
            nc.sync.dma_start(dn[:], dnf[:])
            rdn = stage.tile([4, TT], BF16, tag="rdn", name=f"rdn_{jj}")
            with nc.allow_low_precision(reason="bf16 1/denominator is ample"):
                nc.vector.reciprocal(rdn[:], dn[:])
            rf = stage.tile([1, 4, TT], BF16, tag="rf", name=f"rf_{jj}")
            nc.sync.dma_start(rf[:], rdn[:])
            bc4 = bcp.tile([P, 4, TT], BF16, tag="bc", bufs=2,
                           name=f"bc_{jj}")
            nc.gpsimd.partition_broadcast(bc4[:], rf[:])

            # Previous pair's output projection: ready PE work that fills
            # the gap while this jj's normalization drains on DVE/DMA/GPSIMD
            # (keeps the HAM clock-gate warm).
            if prev_js is not None:
                phase3(prev_js)

            for jloc, jx in enumerate(js):
                jsl = bass.ts(jx, TT)
                for h in range(H_LOC):
                    i = jloc * 2 + h
                    hp = slice(DH * h, DH * h + DH)
                    nc.vector.tensor_mul(attnT[hp, jsl], attnT[hp, jsl],
                                         bc4[hp, i, :])
            prev_js = js
        phase3(prev_js)


_NC_CACHE = None


def _get_nc():
    global _NC_CACHE
    if _NC_CACHE is None:
        nc = bacc.Bacc("TRN2", target_bir_lowering=False, debug=False,
                       num_devices=N_CORES)
        with tile.TileContext(nc) as tc:
            _body(tc)
        nc.compile()
        _NC_CACHE = nc
    return _NC_CACHE


def _in_maps(x, W_Q, W_K, W_V, W_O):
    bf16 = ml_dtypes.bfloat16
    xT = np.ascontiguousarray(
        np.asarray(x, dtype=np.float32).reshape(T, D).T).astype(bf16)
    W_Q = np.asarray(W_Q, dtype=np.float32).astype(bf16)
    W_K = np.asarray(W_K, dtype=np.float32).astype(bf16)
    W_V = np.asarray(W_V, dtype=np.float32).astype(bf16)
    W_O = np.asarray(W_O, dtype=np.float32).astype(bf16)
    maps = []
    for i in range(N_CORES):
        sl = slice(P * i, P * i + P)
        maps.append({
            "xT": xT,
            "wq": np.ascontiguousarray(W_Q[:, sl]),
            "wk": np.ascontiguousarray(W_K[:, sl]),
            "wv": np.ascontiguousarray(W_V[:, sl]),
            "wo": np.ascontiguousarray(W_O[sl, :]),
        })
    return maps


def _gather(results):
    acc = np.zeros([D, T], np.float32)
    for r in results:
        acc += np.asarray(r["outT"]).astype(np.float32)
    return np.ascontiguousarray(acc.T).reshape(B, S, D)


def kernel(x, W_Q, W_K, W_V, W_O):
    nc = _get_nc()
    res = run_bass_kernel_spmd(nc, _in_maps(x, W_Q, W_K, W_V, W_O),
                               core_ids=list(range(N_CORES)))
    return _gather(res.results)


def kernel_profiled(x, W_Q, W_K, W_V, W_O):
    """Like kernel() but with NTFF tracing; returns (output, exec_time_ns)."""
    nc = _get_nc()
    res = run_bass_kernel_spmd(nc, _in_maps(x, W_Q, W_K, W_V, W_O),
                               core_ids=list(range(N_CORES)), trace=True)
    return _gather(res.results), res.exec_time_ns
